# revision 34
# baseline (speedup 1.0000x reference)
"""Trainium2 Bass kernel for nn_Attention_4063039062503.

Reference (per batch b, C=128 channels, N=4096 points):
    q = W1 @ x + b1;  k = W2 @ x + b2          # [C, N]
    s[n, m] = q[:, n] . k[:, m]                # [N, N]
    a = softmax(s, axis=m)
    out = relu(x + x @ a.T)

Math restructure (the projections collapse into one tiny GEMM):
    KtQ = x_K^T (W2^T W1) x_q + u 1^T + 1 v^T + const,  u = x_K^T (W2^T b1)
    The v/const terms are constant over keys -> cancel in softmax.
    So  S_eff^T = x_K^T Z'   with   Z' = A^T... Z' = (W2^T W1) x_q + w 1^T,
    w = W2^T b1.  Host precomputes A = W1^T W2 (lhsT of the Z-proj) and w;
    the device does Z' = matmul(A, x_q) + w (bias folded into the PSUM
    evacuation) and never touches W1/W2/b1/b2 again.  exp() then needs only
    a constant -30 shift -> the ACT engine does nothing but 64 pure exps.

Sharding: 8 cores, core i -> batch i//2, query half i%2 (2048 queries),
full 4096 keys local (no collectives).  Keys are ROTATED per core so the
query half is always columns 0:2048 -> one ascending DMA stream feeds the
Z-projection and the early S-tiles.

Per-core pipeline (flash-attention style, flattened 64-iteration stream):
  - S^T tile [m=128, 1024 q] per (pass, m-tile) on TensorE in fp16
  - exp(s - 30) on ScalarE PSUM->SBUF, bf16 out; ACT runs back-to-back
    (steady-state cadence ~1.01-1.21us/m-tile, ACT/PE co-bound)
  - O[c, q] += xT[m-tile] @ E on TensorE bf16, fp32 PSUM accumulation,
    lagged DLAG=3 m-tiles behind the exp stream (DLAG=2 couples PE<->ACT
    through semaphore latency and costs ~15%)
  - row-sums: DVE bf16 accumulate + 3-stage ones-matmul (stages 2-3 read
    the last two E-tiles directly so the reciprocal starts immediately)
  - pass-0 uses ps_o for O and ps_r for rowsum; pass-1 SWAPS them (O in
    the 2-bank [C,1024] r-slot, rowsums in the o-slots) so pass-1's first
    O-matmul never waits on pass-0's tail reads
  - tail: reciprocal_approx_fast + normalize + residual on DVE; final
    relu on ACT (free after the last exp), DMA out
  - startup: warmups interleaved with the Z-projection halves; xk/xt are
    split into early/bulk SBUF tiles because DMA-completion waits are
    whole-tile and per-queue (a reader of any chunk waits for the queue's
    full counter); bulk transfers must stay on the SP queue (the gpsimd /
    ACT DMA paths are ~2x slower for the same bytes)
"""
from contextlib import ExitStack

import numpy as np
import ml_dtypes

import concourse.tile as tile
from concourse import bacc, mybir
from concourse.bass_utils import run_bass_kernel_spmd

B = 4
C = 128
N = 4096            # keys per batch
NQ = 2048           # queries per core
PW = 1024           # queries per pass
MT = 128            # m (key) tile
N_MT = N // MT      # 32
N_P = NQ // PW      # 2 passes
TOT = N_P * N_MT    # 64 global iterations
DLAG = 3            # O-matmul lag (in m-tiles) behind the S/exp stream
MMF = 512           # max matmul free size (1024 is rejected by the ISA:
                    # a matmul's PSUM output cannot cross a 2KB bank)

F32 = mybir.dt.float32
F16 = mybir.dt.float16
BF16 = mybir.dt.bfloat16
AF = mybir.ActivationFunctionType
ALU = mybir.AluOpType


def build_nc():
    nc = bacc.Bacc("TRN2", target_bir_lowering=False, debug=False, num_devices=8)
    # cols 0..C-1 = A = W1^T W2; col C = w = W2^T b1 (fp16 is plenty for w)
    # cols 0..C-1 = W1^T W2 (Z-proj lhsT); col C = w; cols C+2..2C+1 =
    # W2^T W1 (lhsT of the m-tile-0 fast path G0 = (W2^T W1) xk_tile0)
    a_ext = nc.declare_dram_parameter("a16", [C, 2 * C + 2], F16, isOutput=False)
    xk_ext = nc.declare_dram_parameter("xk", [C, N], F16, isOutput=False)
    xt_ext = nc.declare_dram_parameter("xt", [C, N], BF16, isOutput=False)
    out_ext = nc.declare_dram_parameter("out", [C, NQ], F16, isOutput=True)

    def mm(out_ap, lhsT, rhs, start=True, stop=True):
        wtot = out_ap.shape[-1]
        o = 0
        while o < wtot:
            wd = min(MMF, wtot - o)
            nc.tensor.matmul(out_ap[:, o:o + wd], lhsT, rhs[:, o:o + wd],
                             start=start, stop=stop)
            o += wd

    with ExitStack() as ctx:
        tc = ctx.enter_context(tile.TileContext(nc))
        sb1 = ctx.enter_context(tc.tile_pool(name="sb1", bufs=1))
        consts = sb_in = sb_z = sb_e = sb1
        sb2 = ctx.enter_context(tc.tile_pool(name="sb2", bufs=2))
        sb_acc = sb_tail = sb2
        ps_s = ctx.enter_context(tc.tile_pool(name="ps_s", bufs=2, space="PSUM"))
        ps_o = ctx.enter_context(tc.tile_pool(name="ps_o", bufs=2, space="PSUM"))
        ps_r = ctx.enter_context(tc.tile_pool(name="ps_r", bufs=1, space="PSUM"))

        # warm the PE's HAM clock gate (~3.4us of matmul activity) during
        # the input-DMA wait so the main stream runs at 2.4 GHz; its
        # memset goes first so the warmup starts before the first DMA lands
        # memset on GpSimd: its queue starts ~1us earlier than DVE's, so
        # the warmup (and with it the whole PE stream) begins sooner
        wmm = consts.tile([C, 512], BF16, tag="wmm")
        nc.gpsimd.memset(wmm[:], 0.0)
        def warm_mm(k):
            for _ in range(k):
                wps = ps_s.tile([C, PW], F32, tag="s", name="wps")
                nc.tensor.matmul(wps[:, 0:512], wmm[:, 0:C], wmm[:],
                                 start=True, stop=True)
        warm_mm(2)
        ones_bf = consts.tile([C, C], BF16, tag="ones_bf")
        nc.vector.memset(ones_bf[:], 1.0)
        shift = consts.tile([C, 1], F32, tag="shift")
        nc.vector.memset(shift[:], -30.0)
        zero0 = consts.tile([C, 1], F32, tag="zero0")
        nc.vector.memset(zero0[:], 0.0)
        # warm the exp table early (ACT_TABLE_LOAD ~2.7us)
        warm = consts.tile([1, 16], F32, tag="warm")
        nc.vector.memset(warm[:], 0.0)
        warm_o = consts.tile([1, 16], F32, tag="warm_o")
        nc.scalar.activation(warm_o[:], warm[:], AF.Exp, bias=zero0[0:1, 0:1])

        a16 = sb_in.tile([C, 2 * C + 2], F16, tag="a16")
        # xk/xt split into an early tile (first 8 m-tiles + queries) and a
        # bulk tile: dependency tracking is whole-tile, so a single tile
        # would make every reader wait for the LAST of its DMA chunks
        # xk_a further split in two: Z0a gates on just 0.25MB landing
        # (the transfer time under 8-core HBM contention IS the startup)
        xk_a1 = sb_in.tile([C, 512], F16, tag="xk_a1")
        xk_a2 = sb_in.tile([C, 512], F16, tag="xk_a2")
        xk_b = sb_in.tile([C, N - PW], F16, tag="xk_b")
        xt_a = sb_in.tile([C, PW], BF16, tag="xt_a")
        xt_b = sb_in.tile([C, N - PW], BF16, tag="xt_b")
        # residual x_q recovered from the fp16 xk upload (saves a 1MB DMA;
        # fp16 residual error ~1e-3 abs vs the 2e-2*scale gate)
        xq32 = sb_in.tile([C, NQ], F32, tag="xq32")

        def xk_tile(mt):
            if mt < 4:
                return xk_a1[:, mt * MT:(mt + 1) * MT]
            if mt < 8:
                return xk_a2[:, (mt - 4) * MT:(mt - 3) * MT]
            return xk_b[:, (mt - PW // MT) * MT:(mt - PW // MT + 1) * MT]

        def xt_tile(mt):
            if mt < PW // MT:
                return xt_a[:, mt * MT:(mt + 1) * MT]
            return xt_b[:, (mt - PW // MT) * MT:(mt - PW // MT + 1) * MT]
        zt = sb_z.tile([C, NQ], F16, tag="zt")
        e_stage = sb_e.tile([C, N_MT * PW], BF16, tag="e")

        # input DMAs, gating-first order: the first S-matmul half needs
        # only xk[:, 0:512] + a16 + w; xt0 issues from the GpSimd queue so
        # its descriptor generation overlaps SP's
        # ONLY the gating transfers ride the SP DMA queue: completions on a
        # queue are out-of-order, so a consumer must wait for the queue's
        # whole counter -- keeping the bulk transfers on the GpSimd queue
        # means the Z/S prologue only waits for these three small DMAs
        # all transfers on the SP hwdge queue (ACT-queue gens are slow for
        # big DMAs, gpsimd is SWDGE = slower still); gating ones first so
        # the serial descriptor gens start their transfers earliest
        nc.sync.dma_start(a16[:], a_ext[:])
        nc.sync.dma_start(xk_a1[:], xk_ext[:, 0:512])
        nc.gpsimd.dma_start(xt_a[:], xt_ext[:, 0:PW])
        # bulk in four chunks: shorter bursts interleave better with the
        # other seven cores' concurrent input DMAs (measured: the merged
        # 2-way variant pushed all-DMA-complete from ~12us to ~15us)
        nc.sync.dma_start(xk_b[:, 0:PW], xk_ext[:, PW:2 * PW])
        nc.sync.dma_start(xt_b[:, 0:PW], xt_ext[:, PW:2 * PW])
        nc.sync.dma_start(xk_b[:, PW:N - PW], xk_ext[:, 2 * PW:N])
        nc.sync.dma_start(xt_b[:, PW:N - PW], xt_ext[:, 2 * PW:N])
        # xk_a2 last: keeps it out of the Z0a gating wait set; its
        # consumers (S-tiles 4-7) run ~7us after it lands
        nc.sync.dma_start(xk_a2[:], xk_ext[:, 512:PW])
        # fp32 copy of the w column for use as evac bias (DVE scalars and
        # ACT bias want fp32)
        wf32 = consts.tile([C, 1], F32, tag="wf32")
        nc.vector.tensor_copy(wf32[:], a16[:, C:C + 1])

        def zproj(j, split_evac):
            # Z'[:, j*PW:(j+1)*PW] = A^T... = (W2^T W1) x_q + w 1^T
            zp = ps_r.tile([C, PW], F32, tag="r")
            rhs = xk_a[:] if j == 0 else xk_b[:, 0:PW]
            mm(zp[:], a16[:, 0:C], rhs)
            dst0 = zt[:, j * PW:j * PW + 512]
            dst1 = zt[:, j * PW + 512:(j + 1) * PW]
            if split_evac:
                # first half on ACT: it is ready right after the first
                # Z-matmul, so both evacs run in parallel
                nc.scalar.activation(dst0, zp[:, 0:512], AF.Identity,
                                     bias=wf32[:, 0:1])
            else:
                nc.vector.tensor_scalar(out=dst0, in0=zp[:, 0:512],
                                        scalar1=wf32[:, 0:1], scalar2=None,
                                        op0=ALU.add)
            nc.vector.tensor_scalar(out=dst1, in0=zp[:, 512:PW],
                                    scalar1=wf32[:, 0:1], scalar2=None,
                                    op0=ALU.add)

        # m-tile-0 fast path: S(0) = G0^T x_q with G0 = (W2^T W1) xk_tile0
        # (skips the Z-evac on the first-exp critical chain); the u-bias for
        # tile 0 comes from a 1-column matmul into the exp bias
        y0 = ps_o.tile([C, MT], F32, tag="o", name="y0")
        nc.tensor.matmul(y0[:], a16[:, C + 2:2 * C + 2], xk_a1[:, 0:MT],
                         start=True, stop=True)
        y0t = sb_z.tile([C, MT], F16, tag="y0t")
        nc.scalar.activation(y0t[:], y0[:], AF.Identity, bias=zero0[:, 0:1])
        u0 = ps_s.tile([C, 8], F32, tag="s", name="u0")
        nc.tensor.matmul(u0[:, 0:1], xk_a1[:, 0:MT], a16[:, C:C + 1],
                         start=True, stop=True)
        shift0 = consts.tile([C, 1], F32, tag="shift0")
        nc.vector.tensor_scalar(out=shift0[:], in0=u0[:, 0:1],
                                scalar1=-30.0, scalar2=None, op0=ALU.add)

        # Z0 split into halves in SEPARATE psum tiles (a shared tile would
        # serialize the evacs on whole-tile deps), interleaved with warmups:
        # each half's evac (ACT for the first, DVE for the second) overlaps
        # the next warmup, and the first S-matmul half starts as soon as
        # zt[:, 0:512] exists
        zp0a = ps_r.tile([C, 512], F32, tag="r", name="zp0a")
        nc.tensor.matmul(zp0a[:], a16[:, 0:C], xk_a1[:], start=True, stop=True)
        nc.scalar.activation(zt[:, 0:512], zp0a[:], AF.Identity,
                             bias=wf32[:, 0:1])
        warm_mm(1)
        zp0b = ps_o.tile([C, 512], F32, tag="o", name="zp0b")
        nc.tensor.matmul(zp0b[:], a16[:, 0:C], xk_a2[:], start=True, stop=True)
        nc.vector.tensor_scalar(out=zt[:, 512:PW], in0=zp0b[:],
                                scalar1=wf32[:, 0:1], scalar2=None,
                                op0=ALU.add)
        warm_mm(2)

        acc = [None] * N_P
        o_t = [None] * N_P      # pass 0: (o_psA, o_psB); pass 1: [C,1024]
        r_t = [None] * N_P      # pass 0: [C,1024];       pass 1: (rA, rB)

        def do_o(gg):
            p, mt = divmod(gg, N_MT)
            st = (mt == 0)
            sp = (mt == N_MT - 1)
            if p == 0:
                if st:
                    o_t[0] = (ps_o.tile([C, 512], F32, tag="o", name="o0a"),
                              ps_o.tile([C, 512], F32, tag="o", name="o0b"))
                for j in range(2):
                    nc.tensor.matmul(
                        o_t[0][j][:], xt_tile(mt),
                        e_stage[:, mt * PW + j * 512:mt * PW + (j + 1) * 512],
                        start=st, stop=sp)
            else:
                if st:
                    o_t[1] = ps_r.tile([C, PW], F32, tag="r", name="o1")
                mm(o_t[1][:], xt_tile(mt),
                   e_stage[:, mt * PW:(mt + 1) * PW], start=st, stop=sp)

        def rstage(p, stage):
            # 3-stage row-sum: stage 0 reads acc (complete through m-tile
            # 29), stages 1-2 read the last two E-tiles straight from the
            # stage buffer so the reciprocal can start right after the
            # final exp of the pass
            st = (stage == 0)
            sp = (stage == 2)
            if stage == 0:
                rhs = acc[p][:]
            else:
                emt = N_MT - 3 + stage  # 30, 31
                rhs = e_stage[:, emt * PW:(emt + 1) * PW]
            if p == 0:
                if st:
                    r_t[0] = ps_r.tile([C, PW], F32, tag="r", name="r0")
                mm(r_t[0][:], ones_bf[:], rhs, start=st, stop=sp)
            else:
                if st:
                    r_t[1] = (ps_o.tile([C, 512], F32, tag="o", name="r1a"),
                              ps_o.tile([C, 512], F32, tag="o", name="r1b"))
                for j in range(2):
                    nc.tensor.matmul(r_t[1][j][:], ones_bf[:],
                                     rhs[:, j * 512:(j + 1) * 512],
                                     start=st, stop=sp)

        def tail(p):
            # bc = 1/rowsum; out = relu(O*bc + x); pass-0 all on DVE
            # (non-critical, runs under pass-1's stream); pass-1 interleaved
            # per half across DVE/GpSimd/ACT to shorten the serial chain
            bc = sb_tail.tile([C, PW], F32, tag="bc")
            t2 = sb_tail.tile([C, PW], F32, tag="t2")
            t3 = sb_tail.tile([C, PW], F32, tag="t3")
            oo = sb_tail.tile([C, PW], F16, tag="oo")
            if p == 0:
                nc.vector.reciprocal_approx_fast(bc[:], r_t[0][:])
                for j in range(2):
                    sl = slice(j * 512, (j + 1) * 512)
                    nc.vector.tensor_tensor(t2[:, sl], o_t[0][j][:],
                                            bc[:, sl], op=ALU.mult)
                    nc.vector.tensor_tensor(t3[:, sl], t2[:, sl],
                                            xq32[:, j * 512:(j + 1) * 512],
                                            op=ALU.add)
                    nc.vector.tensor_scalar_max(oo[:, sl], t3[:, sl], 0.0)
                    nc.sync.dma_start(out_ext[:, j * 512:(j + 1) * 512],
                                      oo[:, sl])
                return
            slA = slice(0, 512)
            slB = slice(512, PW)
            # half A: recip+mult on DVE, residual-add on GpSimd, relu on ACT
            nc.vector.reciprocal_approx_fast(bc[:, slA], r_t[1][0][:])
            nc.vector.tensor_tensor(t2[:, slA], o_t[1][:, slA], bc[:, slA],
                                    op=ALU.mult)
            nc.gpsimd.tensor_tensor(t3[:, slA], t2[:, slA],
                                    xq32[:, PW:PW + 512], op=ALU.add)
            nc.scalar.activation(oo[:, slA], t3[:, slA], AF.Relu,
                                 bias=zero0[:, 0:1])
            nc.scalar.dma_start(out_ext[:, PW:PW + 512], oo[:, slA])
            # half B: stays on DVE (runs while GpSimd/ACT finish half A)
            nc.vector.reciprocal_approx_fast(bc[:, slB], r_t[1][1][:])
            nc.vector.tensor_tensor(t2[:, slB], o_t[1][:, slB], bc[:, slB],
                                    op=ALU.mult)
            nc.vector.tensor_tensor(t3[:, slB], t2[:, slB],
                                    xq32[:, PW + 512:NQ], op=ALU.add)
            nc.vector.tensor_scalar_max(oo[:, slB], t3[:, slB], 0.0)
            nc.sync.dma_start(out_ext[:, PW + 512:NQ], oo[:, slB])

        for g in range(TOT + DLAG):
            p, mt = divmod(g, N_MT)
            if g < TOT:
                s_ps = ps_s.tile([C, PW], F32, tag="s")
                if g == 0:
                    nc.tensor.matmul(s_ps[:, 0:512], y0t[:], xk_a1[:],
                                     start=True, stop=True)
                    nc.tensor.matmul(s_ps[:, 512:PW], y0t[:], xk_a2[:],
                                     start=True, stop=True)
                else:
                    mm(s_ps[:], xk_tile(mt), zt[:, p * PW:(p + 1) * PW])
                e_g = e_stage[:, mt * PW:(mt + 1) * PW]
                if g in (0, TOT - 1):
                    # halves: consumers of each half start half a tile sooner
                    bias0 = shift0 if g == 0 else shift
                    nc.scalar.activation(e_g[:, 0:512], s_ps[:, 0:512],
                                         AF.Exp, bias=bias0[:, 0:1])
                    nc.scalar.activation(e_g[:, 512:PW], s_ps[:, 512:PW],
                                         AF.Exp, bias=bias0[:, 0:1])
                else:
                    nc.scalar.activation(e_g, s_ps[:], AF.Exp,
                                         bias=shift[:, 0:1])
            # row-sum stages for the pass ending at g-?: emitted right after
            # this iteration's S so they never delay the next exp
            for pp in range(N_P):
                base = pp * N_MT + N_MT - 1  # g of the pass's S(31)
                if g == base:
                    rstage(pp, 0)
                elif g == base + 1:
                    rstage(pp, 1)
                elif g == base + 2:
                    rstage(pp, 2)
            if g in (1, 2):
                # Z1 halves spread over two iterations: halves the PE-work
                # injection into the early exp stream
                jj = g - 1
                zp1 = ps_r.tile([C, 512], F32, tag="r", name=f"zp1{jj}")
                nc.tensor.matmul(zp1[:], a16[:, 0:C],
                                 xk_b[:, jj * 512:(jj + 1) * 512],
                                 start=True, stop=True)
                nc.vector.tensor_scalar(
                    out=zt[:, PW + jj * 512:PW + (jj + 1) * 512],
                    in0=zp1[:], scalar1=wf32[:, 0:1], scalar2=None,
                    op0=ALU.add)
            if g == 12:
                nc.vector.tensor_copy(xq32[:, 0:512], xk_a1[:])
                nc.vector.tensor_copy(xq32[:, 512:PW], xk_a2[:])
            if g == 44:
                nc.vector.tensor_copy(xq32[:, PW:NQ], xk_b[:, 0:PW])
            if g < TOT:
                p, mt = divmod(g, N_MT)
                # DVE row-sum accumulation (m-tiles 0..29; last two are
                # picked up directly by rstages 1-2)
                if mt == 1:
                    acc[p] = sb_acc.tile([C, PW], BF16, tag="acc", name=f"acc{p}")
                    nc.vector.tensor_tensor(acc[p][:], e_stage[:, 0:PW],
                                            e_g, op=ALU.add)
                elif 2 <= mt <= N_MT - 3:
                    nc.vector.tensor_tensor(acc[p][:], acc[p][:], e_g,
                                            op=ALU.add)
            if g >= DLAG and g - DLAG < TOT - 3:
                do_o(g - DLAG)
            elif g == TOT:
                do_o(TOT - 3)
                do_o(TOT - 2)
            elif g == TOT + 1:
                do_o(TOT - 1)
            for pp in range(N_P):
                # after do_o(pp, 31): the O accumulator is complete
                if g == pp * N_MT + N_MT - 1 + DLAG:
                    tail(pp)

    nc.compile()
    return nc


_NC_CACHE = None


def _get_nc():
    global _NC_CACHE
    if _NC_CACHE is None:
        _NC_CACHE = build_nc()
    return _NC_CACHE


def make_in_maps(x, W1, b1, W2, b2):
    x = np.asarray(x, np.float32)
    W1 = np.asarray(W1, np.float32)
    b1 = np.asarray(b1, np.float32)
    W2 = np.asarray(W2, np.float32)
    b2 = np.asarray(b2, np.float32)
    A = (W1.T @ W2).astype(np.float16)          # lhsT of the Z-projection
    w = W2.T @ b1                               # folded u-bias
    A16W = np.zeros((C, 2 * C + 2), np.float16)
    A16W[:, :C] = A
    A16W[:, C] = w.astype(np.float16)
    A16W[:, C + 2:2 * C + 2] = (W2.T @ W1).astype(np.float16)
    in_maps = []
    for core in range(8):
        b, h = divmod(core, 2)
        xb = x[b]                               # [128, 4096]
        # rotate keys so this core's query half is columns 0:2048
        xrot = np.concatenate([xb[:, h * NQ:], xb[:, :h * NQ]], axis=1)
        xk16 = xrot.astype(np.float16)
        # xt[m, mt*128 + c] = xrot[c, mt*128 + m]
        xtt = np.ascontiguousarray(
            xrot.T.reshape(N_MT, MT, C).transpose(1, 0, 2).reshape(MT, N_MT * C)
        ).astype(ml_dtypes.bfloat16)
        in_maps.append({"a16": A16W, "xk": xk16, "xt": xtt})
    return in_maps


def run(x, W1, b1, W2, b2, trace=False):
    nc = _get_nc()
    in_maps = make_in_maps(x, W1, b1, W2, b2)
    last_err = None
    for _attempt in range(3):
        try:
            res = run_bass_kernel_spmd(nc, in_maps, core_ids=list(range(8)),
                                       trace=trace)
            break
        except Exception as e:  # transient NRT/device errors: retry
            last_err = e
    else:
        raise last_err
    out = np.empty((B, C, N), np.float32)
    for core in range(8):
        b, h = divmod(core, 2)
        out[b][:, h * NQ:(h + 1) * NQ] = \
            res.results[core]["out"].astype(np.float32)
    return out, res


def kernel(x, W1, b1, W2, b2):
    out, _ = run(x, W1, b1, W2, b2, trace=False)
    return out


# revision 35
# speedup vs baseline: 1.0393x; 1.0393x over previous
"""Trainium2 Bass kernel for nn_Attention_4063039062503.

Reference (per batch b, C=128 channels, N=4096 points):
    q = W1 @ x + b1;  k = W2 @ x + b2          # [C, N]
    s[n, m] = q[:, n] . k[:, m]                # [N, N]
    a = softmax(s, axis=m)
    out = relu(x + x @ a.T)

Math restructure (the projections collapse into one tiny GEMM):
    KtQ = x_K^T (W2^T W1) x_q + u 1^T + 1 v^T + const,  u = x_K^T (W2^T b1)
    The v/const terms are constant over keys -> cancel in softmax.
    So  S_eff^T = x_K^T Z'   with   Z' = A^T... Z' = (W2^T W1) x_q + w 1^T,
    w = W2^T b1.  Host precomputes A = W1^T W2 (lhsT of the Z-proj) and w;
    the device does Z' = matmul(A, x_q) + w (bias folded into the PSUM
    evacuation) and never touches W1/W2/b1/b2 again.  exp() then needs only
    a constant -30 shift -> the ACT engine does nothing but 64 pure exps.

Sharding: 8 cores, core i -> batch i//2, query half i%2 (2048 queries),
full 4096 keys local (no collectives).  Keys are ROTATED per core so the
query half is always columns 0:2048 -> one ascending DMA stream feeds the
Z-projection and the early S-tiles.

Per-core pipeline (flash-attention style, flattened 64-iteration stream):
  - S^T tile [m=128, 1024 q] per (pass, m-tile) on TensorE in fp16
  - exp(s - 30) on ScalarE PSUM->SBUF, bf16 out; ACT runs back-to-back
    (steady-state cadence ~1.01-1.21us/m-tile, ACT/PE co-bound)
  - O[c, q] += xT[m-tile] @ E on TensorE bf16, fp32 PSUM accumulation,
    lagged DLAG=3 m-tiles behind the exp stream (DLAG=2 couples PE<->ACT
    through semaphore latency and costs ~15%)
  - row-sums: DVE bf16 accumulate + 3-stage ones-matmul (stages 2-3 read
    the last two E-tiles directly so the reciprocal starts immediately)
  - pass-0 uses ps_o for O and ps_r for rowsum; pass-1 SWAPS them (O in
    the 2-bank [C,1024] r-slot, rowsums in the o-slots) so pass-1's first
    O-matmul never waits on pass-0's tail reads
  - tail: reciprocal_approx_fast + normalize + residual on DVE; final
    relu on ACT (free after the last exp), DMA out
  - startup: warmups interleaved with the Z-projection halves; xk/xt are
    split into early/bulk SBUF tiles because DMA-completion waits are
    whole-tile and per-queue (a reader of any chunk waits for the queue's
    full counter); bulk transfers must stay on the SP queue (the gpsimd /
    ACT DMA paths are ~2x slower for the same bytes)
"""
from contextlib import ExitStack

import numpy as np
import ml_dtypes

import concourse.tile as tile
from concourse import bacc, mybir
from concourse.bass_utils import run_bass_kernel_spmd

B = 4
C = 128
N = 4096            # keys per batch
NQ = 2048           # queries per core
PW = 1024           # queries per pass
MT = 128            # m (key) tile
N_MT = N // MT      # 32
N_P = NQ // PW      # 2 passes
TOT = N_P * N_MT    # 64 global iterations
DLAG = 3            # O-matmul lag (in m-tiles) behind the S/exp stream
MMF = 512           # max matmul free size (1024 is rejected by the ISA:
                    # a matmul's PSUM output cannot cross a 2KB bank)

F32 = mybir.dt.float32
F16 = mybir.dt.float16
BF16 = mybir.dt.bfloat16
AF = mybir.ActivationFunctionType
ALU = mybir.AluOpType


def build_nc():
    nc = bacc.Bacc("TRN2", target_bir_lowering=False, debug=False, num_devices=8)
    # cols 0..C-1 = A = W1^T W2; col C = w = W2^T b1 (fp16 is plenty for w)
    # cols 0..C-1 = W1^T W2 (Z-proj lhsT); col C = w; cols C+2..2C+1 =
    # W2^T W1 (lhsT of the m-tile-0 fast path G0 = (W2^T W1) xk_tile0)
    a_ext = nc.declare_dram_parameter("a16", [C, 2 * C + 2], F16, isOutput=False)
    xk_ext = nc.declare_dram_parameter("xk", [C, N], F16, isOutput=False)
    xt_ext = nc.declare_dram_parameter("xt", [C, N], BF16, isOutput=False)
    out_ext = nc.declare_dram_parameter("out", [C, NQ], F16, isOutput=True)

    def mm(out_ap, lhsT, rhs, start=True, stop=True):
        wtot = out_ap.shape[-1]
        o = 0
        while o < wtot:
            wd = min(MMF, wtot - o)
            nc.tensor.matmul(out_ap[:, o:o + wd], lhsT, rhs[:, o:o + wd],
                             start=start, stop=stop)
            o += wd

    with ExitStack() as ctx:
        tc = ctx.enter_context(tile.TileContext(nc))
        sb1 = ctx.enter_context(tc.tile_pool(name="sb1", bufs=1))
        consts = sb_in = sb_z = sb_e = sb1
        sb2 = ctx.enter_context(tc.tile_pool(name="sb2", bufs=2))
        sb_acc = sb_tail = sb2
        ps_s = ctx.enter_context(tc.tile_pool(name="ps_s", bufs=2, space="PSUM"))
        ps_o = ctx.enter_context(tc.tile_pool(name="ps_o", bufs=2, space="PSUM"))
        ps_r = ctx.enter_context(tc.tile_pool(name="ps_r", bufs=1, space="PSUM"))

        # warm the PE's HAM clock gate (~3.4us of matmul activity) during
        # the input-DMA wait so the main stream runs at 2.4 GHz; its
        # memset goes first so the warmup starts before the first DMA lands
        # memset on GpSimd: its queue starts ~1us earlier than DVE's, so
        # the warmup (and with it the whole PE stream) begins sooner
        wmm = consts.tile([C, 512], BF16, tag="wmm")
        nc.gpsimd.memset(wmm[:], 0.0)
        def warm_mm(k):
            for _ in range(k):
                wps = ps_s.tile([C, PW], F32, tag="s", name="wps")
                nc.tensor.matmul(wps[:, 0:512], wmm[:, 0:C], wmm[:],
                                 start=True, stop=True)
        warm_mm(2)
        ones_bf = consts.tile([C, C], BF16, tag="ones_bf")
        nc.vector.memset(ones_bf[:], 1.0)
        shift = consts.tile([C, 1], F32, tag="shift")
        nc.vector.memset(shift[:], -30.0)
        zero0 = consts.tile([C, 1], F32, tag="zero0")
        nc.vector.memset(zero0[:], 0.0)
        # warm the exp table early (ACT_TABLE_LOAD ~2.7us)
        warm = consts.tile([1, 16], F32, tag="warm")
        nc.vector.memset(warm[:], 0.0)
        warm_o = consts.tile([1, 16], F32, tag="warm_o")
        nc.scalar.activation(warm_o[:], warm[:], AF.Exp, bias=zero0[0:1, 0:1])

        a16 = sb_in.tile([C, 2 * C + 2], F16, tag="a16")
        # xk/xt split into an early tile (first 8 m-tiles + queries) and a
        # bulk tile: dependency tracking is whole-tile, so a single tile
        # would make every reader wait for the LAST of its DMA chunks
        # xk_a further split in two: Z0a gates on just 0.25MB landing
        # (the transfer time under 8-core HBM contention IS the startup)
        xk_a1 = sb_in.tile([C, 512], F16, tag="xk_a1")
        xk_a2 = sb_in.tile([C, 512], F16, tag="xk_a2")
        xk_b = sb_in.tile([C, N - PW], F16, tag="xk_b")
        xt_a = sb_in.tile([C, PW], BF16, tag="xt_a")
        xt_b = sb_in.tile([C, N - PW], BF16, tag="xt_b")
        # residual x_q recovered from the fp16 xk upload (saves a 1MB DMA;
        # fp16 residual error ~1e-3 abs vs the 2e-2*scale gate)
        xq32 = sb_in.tile([C, NQ], F32, tag="xq32")

        def xk_tile(mt):
            if mt < 4:
                return xk_a1[:, mt * MT:(mt + 1) * MT]
            if mt < 8:
                return xk_a2[:, (mt - 4) * MT:(mt - 3) * MT]
            return xk_b[:, (mt - PW // MT) * MT:(mt - PW // MT + 1) * MT]

        def xt_tile(mt):
            if mt < PW // MT:
                return xt_a[:, mt * MT:(mt + 1) * MT]
            return xt_b[:, (mt - PW // MT) * MT:(mt - PW // MT + 1) * MT]
        zt = sb_z.tile([C, NQ], F16, tag="zt")
        e_stage = sb_e.tile([C, N_MT * PW], BF16, tag="e")

        # input DMAs, gating-first order: the first S-matmul half needs
        # only xk[:, 0:512] + a16 + w; xt0 issues from the GpSimd queue so
        # its descriptor generation overlaps SP's
        # ONLY the gating transfers ride the SP DMA queue: completions on a
        # queue are out-of-order, so a consumer must wait for the queue's
        # whole counter -- keeping the bulk transfers on the GpSimd queue
        # means the Z/S prologue only waits for these three small DMAs
        # all transfers on the SP hwdge queue (ACT-queue gens are slow for
        # big DMAs, gpsimd is SWDGE = slower still); gating ones first so
        # the serial descriptor gens start their transfers earliest
        nc.sync.dma_start(a16[:], a_ext[:])
        nc.sync.dma_start(xk_a1[:], xk_ext[:, 0:512])
        nc.sync.dma_start(xk_a2[:], xk_ext[:, 512:PW])
        nc.gpsimd.dma_start(xt_a[:], xt_ext[:, 0:PW])
        # bulk in four chunks: shorter bursts interleave better with the
        # other seven cores' concurrent input DMAs (measured: the merged
        # 2-way variant pushed all-DMA-complete from ~12us to ~15us)
        nc.sync.dma_start(xk_b[:, 0:PW], xk_ext[:, PW:2 * PW])
        nc.sync.dma_start(xt_b[:, 0:PW], xt_ext[:, PW:2 * PW])
        nc.sync.dma_start(xk_b[:, PW:N - PW], xk_ext[:, 2 * PW:N])
        nc.sync.dma_start(xt_b[:, PW:N - PW], xt_ext[:, 2 * PW:N])
        # fp32 copy of the w column for use as evac bias (DVE scalars and
        # ACT bias want fp32)
        wf32 = consts.tile([C, 1], F32, tag="wf32")
        nc.vector.tensor_copy(wf32[:], a16[:, C:C + 1])

        def zproj(j, split_evac):
            # Z'[:, j*PW:(j+1)*PW] = A^T... = (W2^T W1) x_q + w 1^T
            zp = ps_r.tile([C, PW], F32, tag="r")
            rhs = xk_a[:] if j == 0 else xk_b[:, 0:PW]
            mm(zp[:], a16[:, 0:C], rhs)
            dst0 = zt[:, j * PW:j * PW + 512]
            dst1 = zt[:, j * PW + 512:(j + 1) * PW]
            if split_evac:
                # first half on ACT: it is ready right after the first
                # Z-matmul, so both evacs run in parallel
                nc.scalar.activation(dst0, zp[:, 0:512], AF.Identity,
                                     bias=wf32[:, 0:1])
            else:
                nc.vector.tensor_scalar(out=dst0, in0=zp[:, 0:512],
                                        scalar1=wf32[:, 0:1], scalar2=None,
                                        op0=ALU.add)
            nc.vector.tensor_scalar(out=dst1, in0=zp[:, 512:PW],
                                    scalar1=wf32[:, 0:1], scalar2=None,
                                    op0=ALU.add)

        # m-tile-0 fast path: S(0) = G0^T x_q with G0 = (W2^T W1) xk_tile0
        # (skips the Z-evac on the first-exp critical chain); the u-bias for
        # tile 0 comes from a 1-column matmul into the exp bias
        y0 = ps_o.tile([C, MT], F32, tag="o", name="y0")
        nc.tensor.matmul(y0[:], a16[:, C + 2:2 * C + 2], xk_a1[:, 0:MT],
                         start=True, stop=True)
        y0t = sb_z.tile([C, MT], F16, tag="y0t")
        nc.scalar.activation(y0t[:], y0[:], AF.Identity, bias=zero0[:, 0:1])
        u0 = ps_s.tile([C, 8], F32, tag="s", name="u0")
        nc.tensor.matmul(u0[:, 0:1], xk_a1[:, 0:MT], a16[:, C:C + 1],
                         start=True, stop=True)
        shift0 = consts.tile([C, 1], F32, tag="shift0")
        nc.vector.tensor_scalar(out=shift0[:], in0=u0[:, 0:1],
                                scalar1=-30.0, scalar2=None, op0=ALU.add)

        # Z0 split into halves in SEPARATE psum tiles (a shared tile would
        # serialize the evacs on whole-tile deps), interleaved with warmups:
        # each half's evac (ACT for the first, DVE for the second) overlaps
        # the next warmup, and the first S-matmul half starts as soon as
        # zt[:, 0:512] exists
        zp0a = ps_r.tile([C, 512], F32, tag="r", name="zp0a")
        nc.tensor.matmul(zp0a[:], a16[:, 0:C], xk_a1[:], start=True, stop=True)
        nc.scalar.activation(zt[:, 0:512], zp0a[:], AF.Identity,
                             bias=wf32[:, 0:1])
        warm_mm(1)
        zp0b = ps_o.tile([C, 512], F32, tag="o", name="zp0b")
        nc.tensor.matmul(zp0b[:], a16[:, 0:C], xk_a2[:], start=True, stop=True)
        nc.vector.tensor_scalar(out=zt[:, 512:PW], in0=zp0b[:],
                                scalar1=wf32[:, 0:1], scalar2=None,
                                op0=ALU.add)
        warm_mm(2)

        acc = [None] * N_P
        o_t = [None] * N_P      # pass 0: (o_psA, o_psB); pass 1: [C,1024]
        r_t = [None] * N_P      # pass 0: [C,1024];       pass 1: (rA, rB)

        def do_o(gg):
            p, mt = divmod(gg, N_MT)
            st = (mt == 0)
            sp = (mt == N_MT - 1)
            if p == 0:
                if st:
                    o_t[0] = (ps_o.tile([C, 512], F32, tag="o", name="o0a"),
                              ps_o.tile([C, 512], F32, tag="o", name="o0b"))
                for j in range(2):
                    nc.tensor.matmul(
                        o_t[0][j][:], xt_tile(mt),
                        e_stage[:, mt * PW + j * 512:mt * PW + (j + 1) * 512],
                        start=st, stop=sp)
            else:
                if st:
                    o_t[1] = ps_r.tile([C, PW], F32, tag="r", name="o1")
                mm(o_t[1][:], xt_tile(mt),
                   e_stage[:, mt * PW:(mt + 1) * PW], start=st, stop=sp)

        def rstage(p, stage):
            # 3-stage row-sum: stage 0 reads acc (complete through m-tile
            # 29), stages 1-2 read the last two E-tiles straight from the
            # stage buffer so the reciprocal can start right after the
            # final exp of the pass
            st = (stage == 0)
            sp = (stage == 2)
            if stage == 0:
                rhs = acc[p][:]
            else:
                emt = N_MT - 3 + stage  # 30, 31
                rhs = e_stage[:, emt * PW:(emt + 1) * PW]
            if p == 0:
                if st:
                    r_t[0] = ps_r.tile([C, PW], F32, tag="r", name="r0")
                mm(r_t[0][:], ones_bf[:], rhs, start=st, stop=sp)
            else:
                if st:
                    r_t[1] = (ps_o.tile([C, 512], F32, tag="o", name="r1a"),
                              ps_o.tile([C, 512], F32, tag="o", name="r1b"))
                for j in range(2):
                    nc.tensor.matmul(r_t[1][j][:], ones_bf[:],
                                     rhs[:, j * 512:(j + 1) * 512],
                                     start=st, stop=sp)

        def tail(p):
            # bc = 1/rowsum; out = relu(O*bc + x); pass-0 all on DVE
            # (non-critical, runs under pass-1's stream); pass-1 interleaved
            # per half across DVE/GpSimd/ACT to shorten the serial chain
            bc = sb_tail.tile([C, PW], F32, tag="bc")
            t2 = sb_tail.tile([C, PW], F32, tag="t2")
            t3 = sb_tail.tile([C, PW], F32, tag="t3")
            oo = sb_tail.tile([C, PW], F16, tag="oo")
            if p == 0:
                nc.vector.reciprocal_approx_fast(bc[:], r_t[0][:])
                for j in range(2):
                    sl = slice(j * 512, (j + 1) * 512)
                    nc.vector.tensor_tensor(t2[:, sl], o_t[0][j][:],
                                            bc[:, sl], op=ALU.mult)
                    nc.vector.tensor_tensor(t3[:, sl], t2[:, sl],
                                            xq32[:, j * 512:(j + 1) * 512],
                                            op=ALU.add)
                    nc.vector.tensor_scalar_max(oo[:, sl], t3[:, sl], 0.0)
                    nc.sync.dma_start(out_ext[:, j * 512:(j + 1) * 512],
                                      oo[:, sl])
                return
            slA = slice(0, 512)
            slB = slice(512, PW)
            # half A: recip+mult on DVE, residual-add on GpSimd, relu on ACT
            nc.vector.reciprocal_approx_fast(bc[:, slA], r_t[1][0][:])
            nc.vector.tensor_tensor(t2[:, slA], o_t[1][:, slA], bc[:, slA],
                                    op=ALU.mult)
            nc.gpsimd.tensor_tensor(t3[:, slA], t2[:, slA],
                                    xq32[:, PW:PW + 512], op=ALU.add)
            nc.scalar.activation(oo[:, slA], t3[:, slA], AF.Relu,
                                 bias=zero0[:, 0:1])
            nc.scalar.dma_start(out_ext[:, PW:PW + 512], oo[:, slA])
            # half B: stays on DVE (runs while GpSimd/ACT finish half A)
            nc.vector.reciprocal_approx_fast(bc[:, slB], r_t[1][1][:])
            nc.vector.tensor_tensor(t2[:, slB], o_t[1][:, slB], bc[:, slB],
                                    op=ALU.mult)
            nc.vector.tensor_tensor(t3[:, slB], t2[:, slB],
                                    xq32[:, PW + 512:NQ], op=ALU.add)
            nc.vector.tensor_scalar_max(oo[:, slB], t3[:, slB], 0.0)
            nc.sync.dma_start(out_ext[:, PW + 512:NQ], oo[:, slB])

        for g in range(TOT + DLAG):
            p, mt = divmod(g, N_MT)
            if g < TOT:
                s_ps = ps_s.tile([C, PW], F32, tag="s")
                if g == 0:
                    nc.tensor.matmul(s_ps[:, 0:512], y0t[:], xk_a1[:],
                                     start=True, stop=True)
                    nc.tensor.matmul(s_ps[:, 512:PW], y0t[:], xk_a2[:],
                                     start=True, stop=True)
                else:
                    mm(s_ps[:], xk_tile(mt), zt[:, p * PW:(p + 1) * PW])
                e_g = e_stage[:, mt * PW:(mt + 1) * PW]
                if g in (0, TOT - 1):
                    # halves: consumers of each half start half a tile sooner
                    bias0 = shift0 if g == 0 else shift
                    nc.scalar.activation(e_g[:, 0:512], s_ps[:, 0:512],
                                         AF.Exp, bias=bias0[:, 0:1])
                    nc.scalar.activation(e_g[:, 512:PW], s_ps[:, 512:PW],
                                         AF.Exp, bias=bias0[:, 0:1])
                else:
                    nc.scalar.activation(e_g, s_ps[:], AF.Exp,
                                         bias=shift[:, 0:1])
            # row-sum stages for the pass ending at g-?: emitted right after
            # this iteration's S so they never delay the next exp
            for pp in range(N_P):
                base = pp * N_MT + N_MT - 1  # g of the pass's S(31)
                if g == base:
                    rstage(pp, 0)
                elif g == base + 1:
                    rstage(pp, 1)
                elif g == base + 2:
                    rstage(pp, 2)
            if g in (1, 2):
                # Z1 halves spread over two iterations: halves the PE-work
                # injection into the early exp stream
                jj = g - 1
                zp1 = ps_r.tile([C, 512], F32, tag="r", name=f"zp1{jj}")
                nc.tensor.matmul(zp1[:], a16[:, 0:C],
                                 xk_b[:, jj * 512:(jj + 1) * 512],
                                 start=True, stop=True)
                nc.vector.tensor_scalar(
                    out=zt[:, PW + jj * 512:PW + (jj + 1) * 512],
                    in0=zp1[:], scalar1=wf32[:, 0:1], scalar2=None,
                    op0=ALU.add)
            if g == 12:
                nc.vector.tensor_copy(xq32[:, 0:512], xk_a1[:])
                nc.vector.tensor_copy(xq32[:, 512:PW], xk_a2[:])
            if g == 44:
                nc.vector.tensor_copy(xq32[:, PW:NQ], xk_b[:, 0:PW])
            if g < TOT:
                p, mt = divmod(g, N_MT)
                # DVE row-sum accumulation (m-tiles 0..29; last two are
                # picked up directly by rstages 1-2)
                if mt == 1:
                    acc[p] = sb_acc.tile([C, PW], BF16, tag="acc", name=f"acc{p}")
                    nc.vector.tensor_tensor(acc[p][:], e_stage[:, 0:PW],
                                            e_g, op=ALU.add)
                elif 2 <= mt <= N_MT - 3:
                    nc.vector.tensor_tensor(acc[p][:], acc[p][:], e_g,
                                            op=ALU.add)
            if g >= DLAG and g - DLAG < TOT - 3:
                do_o(g - DLAG)
            elif g == TOT:
                do_o(TOT - 3)
                do_o(TOT - 2)
            elif g == TOT + 1:
                do_o(TOT - 1)
            for pp in range(N_P):
                # after do_o(pp, 31): the O accumulator is complete
                if g == pp * N_MT + N_MT - 1 + DLAG:
                    tail(pp)

    nc.compile()
    return nc


_NC_CACHE = None


def _get_nc():
    global _NC_CACHE
    if _NC_CACHE is None:
        _NC_CACHE = build_nc()
    return _NC_CACHE


def make_in_maps(x, W1, b1, W2, b2):
    x = np.asarray(x, np.float32)
    W1 = np.asarray(W1, np.float32)
    b1 = np.asarray(b1, np.float32)
    W2 = np.asarray(W2, np.float32)
    b2 = np.asarray(b2, np.float32)
    A = (W1.T @ W2).astype(np.float16)          # lhsT of the Z-projection
    w = W2.T @ b1                               # folded u-bias
    A16W = np.zeros((C, 2 * C + 2), np.float16)
    A16W[:, :C] = A
    A16W[:, C] = w.astype(np.float16)
    A16W[:, C + 2:2 * C + 2] = (W2.T @ W1).astype(np.float16)
    in_maps = []
    for core in range(8):
        b, h = divmod(core, 2)
        xb = x[b]                               # [128, 4096]
        # rotate keys so this core's query half is columns 0:2048
        xrot = np.concatenate([xb[:, h * NQ:], xb[:, :h * NQ]], axis=1)
        xk16 = xrot.astype(np.float16)
        # xt[m, mt*128 + c] = xrot[c, mt*128 + m]
        xtt = np.ascontiguousarray(
            xrot.T.reshape(N_MT, MT, C).transpose(1, 0, 2).reshape(MT, N_MT * C)
        ).astype(ml_dtypes.bfloat16)
        in_maps.append({"a16": A16W, "xk": xk16, "xt": xtt})
    return in_maps


def run(x, W1, b1, W2, b2, trace=False):
    nc = _get_nc()
    in_maps = make_in_maps(x, W1, b1, W2, b2)
    last_err = None
    for _attempt in range(3):
        try:
            res = run_bass_kernel_spmd(nc, in_maps, core_ids=list(range(8)),
                                       trace=trace)
            break
        except Exception as e:  # transient NRT/device errors: retry
            last_err = e
    else:
        raise last_err
    out = np.empty((B, C, N), np.float32)
    for core in range(8):
        b, h = divmod(core, 2)
        out[b][:, h * NQ:(h + 1) * NQ] = \
            res.results[core]["out"].astype(np.float32)
    return out, res


def kernel(x, W1, b1, W2, b2):
    out, _ = run(x, W1, b1, W2, b2, trace=False)
    return out
